# revision 16
# baseline (speedup 1.0000x reference)
"""4x4 array-multiplier kernel for Trainium2 (Bass, raw), 8-core SPMD.

The reference nn.Module is a spiking-neuron gate network implementing a
combinational 4x4 binary multiplier: A, B are [N, 4] float32 bit vectors
(LSB first), output is [N, 8] float32 bits of the product.

Design (target_regime: memory -- minimize device HBM traffic):
  Host:   packbits A,B bit-planes -> nibble values a,b in uint8 (1 B/row
          per operand; pure bit-level re-serialization of the same data),
          interleaved per tile as [128, 2, f] (a-chunk then b-chunk per
          partition) so each tile is ONE contiguous-HBM DMA.
  Device: p = a * b, one uint8 tensor_tensor multiply per tile on the
          DVE (fp32 internal, 15*15=225 exact in u8). The product byte
          IS the packed 8 output bits of the reference circuit.
  Host:   unpackbits p -> [N, 8] float32 (exact 0.0/1.0).

Per-core HBM traffic: 1.0 MiB in + 0.5 MiB out. DVE: one 1x-mode TT
per tile, ~(151 + f) cycles @0.96 GHz.

Raw bass (no TileContext): hand-rolled semaphores.
Sync protocol (sound by construction):
  - one semaphore per input DMA, +16 on completion (the 16 SDMA engines
    finish their per-DMA slices OUT OF ORDER, so a shared counter would
    be racy);
  - DVE waits its tile's in-sem >=16, multiplies, tt_sem += 1;
  - each output engine waits tt_sem >= t+1, stores tile t, bumps its own
    out-sem += 16; final wait per output engine keeps the NEFF alive
    until every output byte it issued has its HBM write receipt.

v2 structure vs v1: input DMA issues are split across MULTIPLE idle
engines (scalar/gpsimd/...) so the ~0.65us-per-DMA issue cost and the
per-queue completion cadence parallelize; output DMAs can likewise be
split (sync/tensor). Schedule + engine assignments are config at top.

Per-core layout: R = N/8 = 524288 rows. Tile t covers 128*f
consecutive rows; within a tile partition p owns rows
tile_base + p*f .. +f. Input and output use the same mapping, so the
elementwise result lands back in row order.
"""

import os
import sys
from contextlib import ExitStack

import numpy as np

for _p in ("/opt/trn_rl_repo",):
    if _p not in sys.path and os.path.isdir(_p):
        sys.path.insert(0, _p)

import concourse.bass as bass
from concourse import bacc, mybir
from concourse.bass_utils import run_bass_kernel_spmd

N_FULL = 4 * 1024 * 1024
N_CORES = 8
R = N_FULL // N_CORES           # rows per core = 524288
F_TOTAL = R // 128              # 4096 elements per partition

# ---- tunables ---------------------------------------------------------
SCHEDULE = [512, 1024, 1024, 1024, 512]  # per-partition elems per tile
IN_ENGINES = ["scalar", "gpsimd", "scalar", "gpsimd", "scalar"]
OUT_ENGINES = ["sync", "sync", "sync", "sync", "sync"]
SKIP_CONST_MEMSETS = True   # drop 4 dead framework memsets at NEFF head
RECEIPT_SLACK = 0           # full HBM write-receipt wait (cold-load safe)
# ----------------------------------------------------------------------

assert sum(SCHEDULE) == F_TOTAL
ALU = mybir.AluOpType
U8 = mybir.dt.uint8


def _make_bacc(skip_const_memsets: bool, sem_top: int = 256) -> bacc.Bacc:
    """Construct a Bacc.

    skip_const_memsets: the 4 framework const-AP SBUF memsets (fp32 0/1,
    bf16 1, u8 127) are not emitted -- this kernel never reads a const AP,
    so they are dead instructions at the head of the NEFF.

    sem_top < 256 shrinks the declared kernel semaphore range
    [150, sem_top); the NEFF epilogue's per-sem clear chains scale with
    the declared span, and this kernel only needs ~12 semaphores."""
    patches = []
    if skip_const_memsets:
        iface = bass.BassEitherVectorEngine
        patches.append((iface, "memset", iface.memset))
        iface.memset = lambda self, ap, constant: None
    if sem_top != 256:
        patches.append((bass, "get_kernel_semaphore_range",
                        bass.get_kernel_semaphore_range))
        bass.get_kernel_semaphore_range = lambda: range(150, sem_top)
    try:
        return bacc.Bacc()
    finally:
        for obj, name, orig in patches:
            setattr(obj, name, orig)


def build(rows: int = R, schedule=None, in_engines=None, out_engines=None,
          skip_const_memsets: bool = False, receipt_slack: int = 0,
          sem_top: int = 256, tt_engines=None) -> bass.Bass:
    if schedule is None:
        schedule = SCHEDULE
    if in_engines is None:
        in_engines = IN_ENGINES
    if out_engines is None:
        out_engines = OUT_ENGINES
    assert sum(schedule) * 128 == rows
    T = len(schedule)
    assert len(in_engines) == T and len(out_engines) == T
    nc = _make_bacc(skip_const_memsets, sem_top)
    Ih = nc.declare_dram_parameter("I", [2 * rows], U8, isOutput=False)
    Oh = nc.declare_dram_parameter("O", [rows], U8, isOutput=True)
    with ExitStack() as ctx:
        in_sems = [ctx.enter_context(nc.semaphore(f"in_sem{t}"))
                   for t in range(T)]
        tt_sems = {}
        out_engine_names = sorted(set(out_engines))
        out_sems = {e: ctx.enter_context(nc.semaphore(f"out_sem_{e}"))
                    for e in out_engine_names}
        its = [ctx.enter_context(nc.sbuf_tensor(f"it{t}", [128, 2, f], U8))
               for t, f in enumerate(schedule)]
        ots = [ctx.enter_context(nc.sbuf_tensor(f"ot{t}", [128, f], U8))
               for t, f in enumerate(schedule)]

        base = 0
        out_views = []
        in_targets = []
        for t, f in enumerate(schedule):
            rows_t = 128 * f
            out_views.append(
                Oh[base:base + rows_t].rearrange("(p f) -> p f", p=128))
            eng_spec = in_engines[t]
            if isinstance(eng_spec, (tuple, list)):
                # split the tile across two HWDGE queues by partition halves
                half = 64 * 2 * f
                Iv_lo = Ih[2 * base:2 * base + half].rearrange(
                    "(p c f) -> p c f", p=64, c=2)
                Iv_hi = Ih[2 * base + half:2 * (base + rows_t)].rearrange(
                    "(p c f) -> p c f", p=64, c=2)
                getattr(nc, eng_spec[0]).dma_start(
                    its[t][0:64, :, :], Iv_lo).then_inc(in_sems[t], 16)
                getattr(nc, eng_spec[1]).dma_start(
                    its[t][64:128, :, :], Iv_hi).then_inc(in_sems[t], 16)
                in_targets.append(32)
            else:
                Iv = Ih[2 * base:2 * (base + rows_t)].rearrange(
                    "(p c f) -> p c f", p=128, c=2)
                getattr(nc, eng_spec).dma_start(
                    its[t][:, :, :], Iv).then_inc(in_sems[t], 16)
                in_targets.append(16)
            base += rows_t
        tt_counts = {}
        tt_wait = []
        for t, f in enumerate(schedule):
            te = tt_engines[t] if tt_engines else "vector"
            eng = getattr(nc, te)
            eng.wait_ge(in_sems[t], in_targets[t])
            n = tt_counts.get(te, 0) + 1
            tt_counts[te] = n
            if te not in tt_sems:
                tt_sems[te] = ctx.enter_context(nc.semaphore(f"tt_sem_{te}"))
            eng.tensor_tensor(
                ots[t][:, :], its[t][:, 0, :], its[t][:, 1, :], ALU.mult
            ).then_inc(tt_sems[te], 1)
            tt_wait.append((tt_sems[te], n))
        out_counts = {e: 0 for e in out_engine_names}
        for t, f in enumerate(schedule):
            eng = getattr(nc, out_engines[t])
            eng.wait_ge(*tt_wait[t])
            eng.dma_start(out_views[t], ots[t][:, :]).then_inc(
                out_sems[out_engines[t]], 16)
            out_counts[out_engines[t]] += 1
        for e in out_engine_names:
            target = 16 * out_counts[e] - receipt_slack
            if target > 0:
                getattr(nc, e).wait_ge(out_sems[e], target)
    nc.finalize()
    return nc


def _pack(X: np.ndarray) -> np.ndarray:
    """[N, 4] f32 bit-planes (LSB first) -> [N] u8 nibble values."""
    Xb = np.ascontiguousarray(np.asarray(X), dtype=np.float32).astype(np.uint8)
    return np.packbits(Xb, axis=1, bitorder="little").ravel()


def _interleave(a: np.ndarray, b: np.ndarray, schedule) -> np.ndarray:
    """Per-core [R] a, [R] b -> [2R] tile-interleaved input buffer matching
    the kernel's per-tile [128, 2, f] access pattern."""
    I = np.empty(2 * a.size, dtype=np.uint8)
    base = 0
    for f in schedule:
        rows = 128 * f
        blk = I[2 * base:2 * (base + rows)].reshape(128, 2, f)
        blk[:, 0, :] = a[base:base + rows].reshape(128, f)
        blk[:, 1, :] = b[base:base + rows].reshape(128, f)
        base += rows
    return I


def _run(A: np.ndarray, B: np.ndarray, trace: bool = False, tmpdir: str | None = None,
         schedule=None, in_engines=None, out_engines=None,
         skip_const_memsets: bool = SKIP_CONST_MEMSETS,
         receipt_slack: int = RECEIPT_SLACK,
         sem_top: int = 256, tt_engines=None):
    assert A.shape == (N_FULL, 4) and B.shape == (N_FULL, 4), (A.shape, B.shape)
    if schedule is None:
        schedule = SCHEDULE
    a = _pack(A)
    b = _pack(B)

    nc = build(R, schedule, in_engines, out_engines, skip_const_memsets,
               receipt_slack, sem_top, tt_engines)
    in_maps = [
        {"I": _interleave(a[i * R:(i + 1) * R], b[i * R:(i + 1) * R], schedule)}
        for i in range(N_CORES)
    ]
    kres = run_bass_kernel_spmd(
        nc, in_maps, list(range(N_CORES)), trace=trace, tmpdir=tmpdir
    )
    P = np.empty(N_FULL, dtype=np.uint8)
    for i in range(N_CORES):
        P[i * R:(i + 1) * R] = np.asarray(kres.results[i]["O"]).reshape(-1)
    out = np.unpackbits(P[:, None], axis=1, bitorder="little").astype(np.float32)
    return out, kres


def kernel(A: np.ndarray, B: np.ndarray) -> np.ndarray:
    out, _ = _run(A, B, trace=False)
    return out
